# revision 27
# baseline (speedup 1.0000x reference)
"""Trainium2 Bass kernel for nn_MultiHeadAttention_79130477461654.

The reference einsum "nhqk,nhvd->nhqd" contracts k and v independently, so
out = (sum_k softmax(energy))*(sum_s v) = broadcast(sum_s v) since softmax
rows sum to 1.  With v = split_heads(x @ Wv) and the reference's direct
(n,h,q,d)->(n,s,e) reshape, the full output reduces to

    xs[n]    = sum_s x[n,s,:]                       (1024,)
    Sfull[n] = xs[n] @ Wv                           (1024,)
    T[n,h,:] = tile16(Sfull[n][64h:64h+64]) @ Wo + bo   (16, 1024)
    out[n, 64h+r, :] = T[n,h,:]   for r in 0..63

Sharding: data parallel over batch N=8, one batch per core; Wv/Wo
replicated.  All tensors ship in bf16 and are pre-shuffled on the host
into the exact SBUF tile layouts (pure layout transform: every DMA line
is >=4KB contiguous, ~420GB/s).  All arithmetic runs on-device with f32
PSUM accumulation; end-to-end rel err ~5e-3 vs the 2e-2 gate.  Per-core
HBM traffic: x 2MB + Wv 2MB + Wo 2MB in, out 2MB.

Pipeline: x streams first on both HWDGE queues and reduces on DVE
chasing the DMAs while dummy matmuls warm the PE HAM clock gate; S
accumulates 1024-wide matmuls in wv-arrival order; dbl128[p, 8h+j] =
Sfull[64h + p%64] makes T128[8h+j, :] = T[h, :] land on all 128
partitions; the output expands to full 1024-row form with one-hot sel
matmuls (PSUM alternating between the psO pool and the retired psT
bank) and PSUM->SBUF copies alternating DVE/ACT, then streams out as
four 512KB DMAs.
"""

import numpy as np

N, S, E, H, D = 8, 1024, 1024, 16, 64
NCORES = 8
P = 128  # partitions
REP = P // H  # 8 copies of each head row


def build_nc():
    import concourse.bacc as bacc
    import concourse.mybir as mybir
    from concourse.tile import TileContext

    F32 = mybir.dt.float32
    BF16 = mybir.dt.bfloat16
    nc = bacc.Bacc("TRN2", target_bir_lowering=False, debug=False)

    xd = nc.declare_dram_parameter("x", [P, 8 * E], BF16, isOutput=False)
    wvd = nc.declare_dram_parameter("Wv", [P, 8 * E], BF16, isOutput=False)
    wod = nc.declare_dram_parameter("Wo", [P, 8 * E], BF16, isOutput=False)
    bod = nc.declare_dram_parameter("bo1", [1, E], BF16, isOutput=False)
    ones128d = nc.declare_dram_parameter("ones128", [P, 1], BF16, isOutput=False)
    seld = nc.declare_dram_parameter("sel", [P, 8 * P], BF16, isOutput=False)
    outd = nc.declare_dram_parameter("out", [S, E], BF16, isOutput=True)

    # two HWDGE queues: SP (sync) and ACT (scalar)
    dmae = [nc.sync, nc.scalar]
    # wv/wo quarter-transfer completion order given the queue layout below:
    # q0 gets chunk-pairs (0,1) then (2,3); q1 gets (4,5) then (6,7).
    KORDER = [0, 1, 4, 5, 2, 3, 6, 7]

    with TileContext(nc) as tc:
        with (
            tc.tile_pool(name="xin", bufs=4) as xp,
            tc.tile_pool(name="wv", bufs=4) as wvp,
            tc.tile_pool(name="wo", bufs=4) as wop,
            tc.tile_pool(name="small", bufs=1) as sp,
            tc.tile_pool(name="psA", bufs=1, space="PSUM") as psA,
            tc.tile_pool(name="psS", bufs=1, space="PSUM") as psS,
            tc.tile_pool(name="psT", bufs=1, space="PSUM") as psT,
            tc.tile_pool(name="psO", bufs=1, space="PSUM") as psO,
        ):
            ones_sb = sp.tile([P, 1], BF16)
            dmae[0].dma_start(out=ones_sb[:], in_=ones128d[:])
            bo_sb = sp.tile([1, E], BF16)
            dmae[1].dma_start(out=bo_sb[:], in_=bod[:])

            # ---- x: 4 x 512KB transfers, heads of both queues.  Transfer i
            #      holds rows [256i, 256i+256): line p = rows 256i+2p(+1).
            xt = []
            for i in range(4):
                t = xp.tile([P, 2 * E], BF16)
                dmae[i // 2].dma_start(
                    out=t[:], in_=xd[:, i * 2 * E : (i + 1) * 2 * E]
                )
                xt.append(t)
            # Wv/Wo: 4 x 512KB each, chunk-pairs (0,1)/(2,3) on q0 and
            # (4,5)/(6,7) on q1, so halves of both land early; consumers
            # run in KORDER.  wv tile q holds K-chunks 2q, 2q+1 of Wv;
            # wo tile q likewise (full-width rows).
            wvt, wot = [], []
            for q in range(4):
                pair = [0, 2, 1, 3][q]  # issue order: (0,1) q0, (4,5) q1, ...
                t = wvp.tile([P, 2 * E], BF16, tag=f"wv{pair}")
                dmae[q % 2].dma_start(
                    out=t[:], in_=wvd[:, pair * 2 * E : (pair + 1) * 2 * E]
                )
                wvt.append((pair, t))
            for q in range(4):
                pair = [0, 2, 1, 3][q]
                t = wop.tile([P, 2 * E], BF16, tag=f"wo{pair}")
                dmae[q % 2].dma_start(
                    out=t[:], in_=wod[:, pair * 2 * E : (pair + 1) * 2 * E]
                )
                wot.append((pair, t))
            sel_sb = sp.tile([P, 8 * P], BF16, tag="sel")
            dmae[0].dma_start(out=sel_sb[:], in_=seld[:])
            wvmap = {pair: t for pair, t in wvt}
            womap = {pair: t for pair, t in wot}

            # ---- PE warm-up: dummy 256-wide matmuls tied to each arriving
            #      x tile keep the HAM clock gate at 2.4 GHz through the
            #      whole x-stream window (not just its start).
            ps_warm = psA.tile([1, 256], F32, tag="psw")

            def warm(rhs):
                nc.tensor.matmul(ps_warm[0:1, :], ones_sb[:], rhs, start=True, stop=True)

            for _ in range(4):
                warm(ones_sb[:, 0:1].to_broadcast((P, 256)))
            for i in range(4):
                for w in range(4):
                    warm(xt[i][:, w * 256 : w * 256 + 256])
            ones_r = sp.tile([1, P], BF16, tag="onesr")
            nc.vector.tensor_copy(ones_r[:], ones_sb[0:1, 0:1].to_broadcast((1, P)))

            # ---- DVE reduction tree over the 8 row-chunks of x (bf16 2x),
            #      paired by arrival order (x0,x1 on q0; x2,x3 on q1).
            af = []
            for i in [0, 2, 1, 3]:
                t = sp.tile([P, E], BF16, tag=f"af{i}")
                nc.vector.tensor_add(t[:], xt[i][:, 0:E], xt[i][:, E : 2 * E])
                af.append((i, t))
            afmap = {i: t for i, t in af}
            a01 = sp.tile([P, E], BF16, tag="a01")
            nc.vector.tensor_add(a01[:], afmap[0][:], afmap[2][:])
            a23 = sp.tile([P, E], BF16, tag="a23")
            nc.vector.tensor_add(a23[:], afmap[1][:], afmap[3][:])
            xacc = sp.tile([P, E], BF16, tag="xacc")
            nc.vector.tensor_add(xacc[:], a01[:], a23[:])

            # ---- xsT[p, c] = xs[128c + p]: PE partition reduction of xacc.
            ps_xsT = psA.tile([P, 8], F32, tag="psa")
            for c in range(8):
                nc.tensor.matmul(
                    ps_xsT[:, c : c + 1],
                    xacc[:, c * P : (c + 1) * P],
                    ones_sb[:],
                    start=True,
                    stop=True,
                )
            xsT_b = sp.tile([P, 8], BF16, tag="xsT")
            nc.vector.tensor_copy(xsT_b[:], ps_xsT[:])

            # ---- Sfull row (1, 1024) = xs @ Wv: 1024-wide matmuls in
            #      wv-arrival order.
            ps_S = psS.tile([1, E], F32, tag="pss")
            for idx, k in enumerate(KORDER):
                for half in range(2):
                    sl = slice(half * 512, half * 512 + 512)
                    nc.tensor.matmul(
                        ps_S[0:1, sl],
                        xsT_b[:, k : k + 1],
                        wvmap[k // 2][
                            :, (k % 2) * E + half * 512 : (k % 2) * E + half * 512 + 512
                        ],
                        start=(idx == 0),
                        stop=(idx == 7),
                        skip_group_check=True,
                    )
            srow_b = sp.tile([1, E], BF16, tag="srow")
            nc.vector.tensor_copy(srow_b[0:1, 0:512], ps_S[0:1, 0:512])
            nc.scalar.copy(out=srow_b[0:1, 512:E], in_=ps_S[0:1, 512:E])
            # filler warmups bridge the srow-copy PE gap
            for w in range(4):
                warm(xt[0][:, w * 256 : w * 256 + 256])

            # ---- dbl[p, h] = Sfull[64h + p%64] (N=1 matmuls, disjoint
            #      start/stop groups; heads 0-7 chase the DVE srow half),
            #      then replicate each head column 8x.
            ps_dbl = psA.tile([P, H], F32, tag="psa")
            for h in range(H):
                for q in range(2):
                    nc.tensor.matmul(
                        ps_dbl[q * D : (q + 1) * D, h : h + 1],
                        srow_b[0:1, h * D : (h + 1) * D],
                        ones_sb[0:1, 0:1],
                        start=True,
                        stop=True,
                    )
            dbl_b = sp.tile([P, H], BF16, tag="dbl")
            nc.vector.tensor_copy(dbl_b[:], ps_dbl[:])
            dbl128 = sp.tile([P, P], BF16, tag="dbl128")
            nc.vector.tensor_copy(
                dbl128[:].rearrange("p (h j) -> p h j", j=REP),
                dbl_b[:, :, None].to_broadcast((P, H, REP)),
            )

            # ---- T128[8h+j, :] = T[h, :] = dbl128.T @ Wo + bo: 1024-wide
            #      matmuls in wo-arrival order; bias opens the group.
            ps_T = psT.tile([P, E], F32, tag="pst")
            for half in range(2):
                sl = slice(half * 512, half * 512 + 512)
                nc.tensor.matmul(
                    ps_T[:, sl],
                    ones_r[:],
                    bo_sb[0:1, sl],
                    start=True,
                    stop=False,
                    skip_group_check=True,
                )
            for idx, k in enumerate(KORDER):
                for half in range(2):
                    sl = slice(half * 512, half * 512 + 512)
                    nc.tensor.matmul(
                        ps_T[:, sl],
                        dbl128[:],
                        womap[k // 2][
                            :, (k % 2) * E + half * 512 : (k % 2) * E + half * 512 + 512
                        ],
                        start=False,
                        stop=(idx == 7),
                        skip_group_check=True,
                    )
            T_sb = sp.tile([P, E], BF16, tag="tsb")
            nc.vector.tensor_copy(T_sb[:, 0:512], ps_T[:, 0:512])
            nc.scalar.copy(out=T_sb[:, 512:E], in_=ps_T[:, 512:E])

            # ---- expansion: out chunk j rows 128j+p = T[2j + p//64] via
            #      one-hot sel matmuls (PSUM alternates psO pool / retired
            #      psT bank); each chunk's PSUM->SBUF copy is half-split
            #      across DVE+ACT and streams out as its own 256KB DMA.
            ob = sp.tile([P, 8 * E], BF16, tag="ob")
            outr = outd.rearrange("(j p) e -> j p e", p=P)
            for j in range(8):
                if j % 2 == 0:
                    po = psO.tile([P, E], F32, tag="pso")
                else:
                    po = psT.tile([P, E], F32, tag="pst")
                for half in range(2):
                    sl = slice(half * 512, half * 512 + 512)
                    nc.tensor.matmul(
                        po[:, sl],
                        sel_sb[:, j * P : (j + 1) * P],
                        T_sb[:, sl],
                        start=True,
                        stop=True,
                    )
                nc.vector.tensor_copy(ob[:, j * E : j * E + 512], po[:, 0:512])
                nc.scalar.copy(out=ob[:, j * E + 512 : (j + 1) * E], in_=po[:, 512:E])
                dmae[j % 2].dma_start(
                    out=outr[j], in_=ob[:, j * E : (j + 1) * E]
                )

    nc.compile()
    return nc


_NC_CACHE = None


def make_in_maps(x, Wv, Wo, bo):
    import ml_dtypes

    bf16 = ml_dtypes.bfloat16
    x = np.asarray(x).astype(bf16)
    Wv = np.asarray(Wv).astype(bf16)
    Wo = np.asarray(Wo).astype(bf16)
    # pre-shuffle into SBUF tile layouts (pure layout transforms):
    # x_pre[p, i*2048 + r*1024 + e] = x[n, 256i + 2p + r, e]
    xs_pre = [
        np.ascontiguousarray(
            x[j].reshape(4, P, 2, E).transpose(1, 0, 2, 3).reshape(P, 8 * E)
        )
        for j in range(NCORES)
    ]
    # w_pre[p, k*1024 + e] = W[128k + p, e]
    wv_pre = np.ascontiguousarray(
        Wv.reshape(8, P, E).transpose(1, 0, 2).reshape(P, 8 * E)
    )
    wo_pre = np.ascontiguousarray(
        Wo.reshape(8, P, E).transpose(1, 0, 2).reshape(P, 8 * E)
    )
    bo1 = np.asarray(bo).astype(bf16).reshape(1, E)
    ones128 = np.ones((P, 1), dtype=bf16)
    sel = np.zeros((P, 8 * P), dtype=np.float32)
    for j in range(8):
        for m in range(P):
            sel[16 * j + 8 * (m // D), j * P + m] = 1.0
    sel = sel.astype(bf16)
    return [
        {
            "x": xs_pre[j],
            "Wv": wv_pre,
            "Wo": wo_pre,
            "bo1": bo1,
            "ones128": ones128,
            "sel": sel,
        }
        for j in range(NCORES)
    ]


def kernel(x, Wq=None, Wk=None, Wv=None, Wo=None, bo=None, **_unused):
    from concourse.bass_utils import run_bass_kernel_spmd

    global _NC_CACHE
    if _NC_CACHE is None:
        _NC_CACHE = build_nc()
    nc = _NC_CACHE

    in_maps = make_in_maps(x, Wv, Wo, bo)
    res = run_bass_kernel_spmd(nc, in_maps, core_ids=list(range(NCORES))).results
    return np.stack(
        [np.asarray(res[j]["out"]).astype(np.float32) for j in range(NCORES)], axis=0
    )


# revision 31
# speedup vs baseline: 1.0123x; 1.0123x over previous
"""Trainium2 Bass kernel for nn_MultiHeadAttention_79130477461654.

The reference einsum "nhqk,nhvd->nhqd" contracts k and v independently, so
out = (sum_k softmax(energy))*(sum_s v) = broadcast(sum_s v) since softmax
rows sum to 1.  With v = split_heads(x @ Wv) and the reference's direct
(n,h,q,d)->(n,s,e) reshape, the full output reduces to

    xs[n]    = sum_s x[n,s,:]                       (1024,)
    Sfull[n] = xs[n] @ Wv                           (1024,)
    T[n,h,:] = tile16(Sfull[n][64h:64h+64]) @ Wo + bo   (16, 1024)
    out[n, 64h+r, :] = T[n,h,:]   for r in 0..63

Sharding: data parallel over batch N=8, one batch per core; Wv/Wo
replicated.  All tensors ship in bf16 and are pre-shuffled on the host
into the exact SBUF tile layouts (pure layout transform: every DMA line
is >=4KB contiguous, ~420GB/s).  All arithmetic runs on-device with f32
PSUM accumulation; end-to-end rel err ~5e-3 vs the 2e-2 gate.  Per-core
HBM traffic: x 2MB + Wv 2MB + Wo 2MB in, out 2MB.

Pipeline: x streams first on both HWDGE queues and reduces on DVE
chasing the DMAs while dummy matmuls warm the PE HAM clock gate; S
accumulates 1024-wide matmuls in wv-arrival order; dbl128[p, 8h+j] =
Sfull[64h + p%64] makes T128[8h+j, :] = T[h, :] land on all 128
partitions; the output expands to full 1024-row form with one-hot sel
matmuls (PSUM alternating between the psO pool and the retired psT
bank) and PSUM->SBUF copies alternating DVE/ACT, then streams out as
four 512KB DMAs.
"""

import numpy as np

N, S, E, H, D = 8, 1024, 1024, 16, 64
NCORES = 8
P = 128  # partitions
REP = P // H  # 8 copies of each head row


def build_nc():
    import concourse.bacc as bacc
    import concourse.mybir as mybir
    from concourse.tile import TileContext

    F32 = mybir.dt.float32
    BF16 = mybir.dt.bfloat16
    nc = bacc.Bacc("TRN2", target_bir_lowering=False, debug=False)

    xd = nc.declare_dram_parameter("x", [P, 8 * E], BF16, isOutput=False)
    wvd = nc.declare_dram_parameter("Wv", [P, 8 * E], BF16, isOutput=False)
    wod = nc.declare_dram_parameter("Wo", [P, 8 * E], BF16, isOutput=False)
    bod = nc.declare_dram_parameter("bo1", [1, E], BF16, isOutput=False)
    ones128d = nc.declare_dram_parameter("ones128", [P, 1], BF16, isOutput=False)
    seld = nc.declare_dram_parameter("sel", [P, 8 * P], BF16, isOutput=False)
    outd = nc.declare_dram_parameter("out", [S, E], BF16, isOutput=True)

    # two HWDGE queues: SP (sync) and ACT (scalar)
    dmae = [nc.sync, nc.scalar]
    # wv/wo quarter-transfer completion order given the queue layout below:
    # q0 gets chunk-pairs (0,1) then (2,3); q1 gets (4,5) then (6,7).
    KORDER = [0, 1, 4, 5, 2, 3, 6, 7]

    with TileContext(nc) as tc:
        with (
            tc.tile_pool(name="xin", bufs=4) as xp,
            tc.tile_pool(name="wv", bufs=4) as wvp,
            tc.tile_pool(name="wo", bufs=4) as wop,
            tc.tile_pool(name="small", bufs=1) as sp,
            tc.tile_pool(name="psA", bufs=1, space="PSUM") as psA,
            tc.tile_pool(name="psS", bufs=1, space="PSUM") as psS,
            tc.tile_pool(name="psT", bufs=1, space="PSUM") as psT,
            tc.tile_pool(name="psO", bufs=1, space="PSUM") as psO,
        ):
            with tc.high_priority():
                ones_sb = sp.tile([P, 1], BF16)
                dmae[0].dma_start(out=ones_sb[:], in_=ones128d[:])
                bo_sb = sp.tile([1, E], BF16)
                dmae[1].dma_start(out=bo_sb[:], in_=bod[:])

                # ---- x: 4 x 512KB transfers, heads of both queues (pinned
                #      first via priority).  Transfer i holds rows
                #      [256i, 256i+256): line p = rows 256i+2p(+1).
                xt = []
                for i in range(4):
                    t = xp.tile([P, 2 * E], BF16)
                    dmae[i % 2].dma_start(
                        out=t[:], in_=xd[:, i * 2 * E : (i + 1) * 2 * E]
                    )
                    xt.append(t)
            # Wv/Wo: 4 x 512KB each, chunk-pairs (0,1)/(2,3) on q0 and
            # (4,5)/(6,7) on q1, so halves of both land early; consumers
            # run in KORDER.  wv tile q holds K-chunks 2q, 2q+1 of Wv;
            # wo tile q likewise (full-width rows).
            wvt, wot = [], []
            for q in range(4):
                pair = [0, 2, 1, 3][q]  # issue order: (0,1) q0, (4,5) q1, ...
                t = wvp.tile([P, 2 * E], BF16, tag=f"wv{pair}")
                dmae[q % 2].dma_start(
                    out=t[:], in_=wvd[:, pair * 2 * E : (pair + 1) * 2 * E]
                )
                wvt.append((pair, t))
            for q in range(4):
                pair = [0, 2, 1, 3][q]
                t = wop.tile([P, 2 * E], BF16, tag=f"wo{pair}")
                dmae[q % 2].dma_start(
                    out=t[:], in_=wod[:, pair * 2 * E : (pair + 1) * 2 * E]
                )
                wot.append((pair, t))
            sel_sb = sp.tile([P, 8 * P], BF16, tag="sel")
            dmae[0].dma_start(out=sel_sb[:], in_=seld[:])
            wvmap = {pair: t for pair, t in wvt}
            womap = {pair: t for pair, t in wot}

            # ---- DVE reduction tree over the 8 row-chunks of x (bf16 2x)
            af = []
            for i in range(4):
                t = sp.tile([P, E], BF16, tag=f"af{i}")
                nc.vector.tensor_add(t[:], xt[i][:, 0:E], xt[i][:, E : 2 * E])
                af.append(t)

            # ---- PE warm-up: dummy 256-wide matmuls keep the HAM clock
            #      gate at 2.4 GHz until S starts; the first dozen feed off
            #      the ones column, the rest off the early af0 tile so the
            #      PE FIFO never stalls on a late x transfer.
            ps_warm = psA.tile([1, 256], F32, tag="psw")

            def warm(rhs):
                nc.tensor.matmul(ps_warm[0:1, :], ones_sb[:], rhs, start=True, stop=True)

            for _ in range(12):
                warm(ones_sb[:, 0:1].to_broadcast((P, 256)))
            for w in range(22):
                warm(af[0][:, (w % 4) * 256 : (w % 4) * 256 + 256])
            ones_r = sp.tile([1, P], BF16, tag="onesr")
            nc.vector.tensor_copy(ones_r[:], ones_sb[0:1, 0:1].to_broadcast((1, P)))

            a01 = sp.tile([P, E], BF16, tag="a01")
            nc.vector.tensor_add(a01[:], af[0][:], af[1][:])
            a23 = sp.tile([P, E], BF16, tag="a23")
            nc.vector.tensor_add(a23[:], af[2][:], af[3][:])
            xacc = sp.tile([P, E], BF16, tag="xacc")
            nc.vector.tensor_add(xacc[:], a01[:], a23[:])

            # ---- xsT[p, c] = xs[128c + p]: PE partition reduction of xacc.
            ps_xsT = psA.tile([P, 8], F32, tag="psa")
            for c in range(8):
                nc.tensor.matmul(
                    ps_xsT[:, c : c + 1],
                    xacc[:, c * P : (c + 1) * P],
                    ones_sb[:],
                    start=True,
                    stop=True,
                )
            xsT_b = sp.tile([P, 8], BF16, tag="xsT")
            nc.vector.tensor_copy(xsT_b[:], ps_xsT[:])

            # ---- Sfull row (1, 1024) = xs @ Wv: 1024-wide matmuls in
            #      wv-arrival order.
            ps_S = psS.tile([1, E], F32, tag="pss")
            for idx, k in enumerate(KORDER):
                for half in range(2):
                    sl = slice(half * 512, half * 512 + 512)
                    nc.tensor.matmul(
                        ps_S[0:1, sl],
                        xsT_b[:, k : k + 1],
                        wvmap[k // 2][
                            :, (k % 2) * E + half * 512 : (k % 2) * E + half * 512 + 512
                        ],
                        start=(idx == 0),
                        stop=(idx == 7),
                        skip_group_check=True,
                    )
            srow_b = sp.tile([1, E], BF16, tag="srow")
            nc.vector.tensor_copy(srow_b[0:1, 0:512], ps_S[0:1, 0:512])
            nc.scalar.copy(out=srow_b[0:1, 512:E], in_=ps_S[0:1, 512:E])
            # filler warmups bridge the srow-copy PE gap
            for w in range(4):
                warm(af[0][:, w * 256 : w * 256 + 256])

            # ---- dbl[p, h] = Sfull[64h + p%64] (N=1 matmuls, disjoint
            #      start/stop groups; heads 0-7 chase the DVE srow half),
            #      then replicate each head column 8x.
            ps_dbl = psA.tile([P, H], F32, tag="psa")
            for h in range(H):
                for q in range(2):
                    nc.tensor.matmul(
                        ps_dbl[q * D : (q + 1) * D, h : h + 1],
                        srow_b[0:1, h * D : (h + 1) * D],
                        ones_sb[0:1, 0:1],
                        start=True,
                        stop=True,
                    )
            dbl_b = sp.tile([P, H], BF16, tag="dbl")
            nc.vector.tensor_copy(dbl_b[:], ps_dbl[:])
            dbl128 = sp.tile([P, P], BF16, tag="dbl128")
            nc.vector.tensor_copy(
                dbl128[:].rearrange("p (h j) -> p h j", j=REP),
                dbl_b[:, :, None].to_broadcast((P, H, REP)),
            )

            # ---- T128[8h+j, :] = T[h, :] = dbl128.T @ Wo + bo: 1024-wide
            #      matmuls in wo-arrival order; bias opens the group.
            ps_T = psT.tile([P, E], F32, tag="pst")
            for half in range(2):
                sl = slice(half * 512, half * 512 + 512)
                nc.tensor.matmul(
                    ps_T[:, sl],
                    ones_r[:],
                    bo_sb[0:1, sl],
                    start=True,
                    stop=False,
                    skip_group_check=True,
                )
            for idx, k in enumerate(KORDER):
                for half in range(2):
                    sl = slice(half * 512, half * 512 + 512)
                    nc.tensor.matmul(
                        ps_T[:, sl],
                        dbl128[:],
                        womap[k // 2][
                            :, (k % 2) * E + half * 512 : (k % 2) * E + half * 512 + 512
                        ],
                        start=False,
                        stop=(idx == 7),
                        skip_group_check=True,
                    )
            T_sb = sp.tile([P, E], BF16, tag="tsb")
            nc.vector.tensor_copy(T_sb[:, 0:512], ps_T[:, 0:512])
            nc.scalar.copy(out=T_sb[:, 512:E], in_=ps_T[:, 512:E])

            # ---- expansion: out chunk j rows 128j+p = T[2j + p//64] via
            #      one-hot sel matmuls (PSUM alternates psO pool / retired
            #      psT bank); each chunk's PSUM->SBUF copy is half-split
            #      across DVE+ACT and streams out as its own 256KB DMA.
            ob = sp.tile([P, 8 * E], BF16, tag="ob")
            outr = outd.rearrange("(j p) e -> j p e", p=P)
            for j in range(8):
                if j % 2 == 0:
                    po = psO.tile([P, E], F32, tag="pso")
                else:
                    po = psT.tile([P, E], F32, tag="pst")
                for half in range(2):
                    sl = slice(half * 512, half * 512 + 512)
                    nc.tensor.matmul(
                        po[:, sl],
                        sel_sb[:, j * P : (j + 1) * P],
                        T_sb[:, sl],
                        start=True,
                        stop=True,
                    )
                nc.vector.tensor_copy(ob[:, j * E : j * E + 512], po[:, 0:512])
                nc.scalar.copy(out=ob[:, j * E + 512 : (j + 1) * E], in_=po[:, 512:E])
                dmae[0].dma_start(out=outr[j], in_=ob[:, j * E : (j + 1) * E])

    nc.compile()
    return nc


_NC_CACHE = None


def make_in_maps(x, Wv, Wo, bo):
    import ml_dtypes

    bf16 = ml_dtypes.bfloat16
    x = np.asarray(x).astype(bf16)
    Wv = np.asarray(Wv).astype(bf16)
    Wo = np.asarray(Wo).astype(bf16)
    # pre-shuffle into SBUF tile layouts (pure layout transforms):
    # x_pre[p, i*2048 + r*1024 + e] = x[n, 256i + 2p + r, e]
    xs_pre = [
        np.ascontiguousarray(
            x[j].reshape(4, P, 2, E).transpose(1, 0, 2, 3).reshape(P, 8 * E)
        )
        for j in range(NCORES)
    ]
    # w_pre[p, k*1024 + e] = W[128k + p, e]
    wv_pre = np.ascontiguousarray(
        Wv.reshape(8, P, E).transpose(1, 0, 2).reshape(P, 8 * E)
    )
    wo_pre = np.ascontiguousarray(
        Wo.reshape(8, P, E).transpose(1, 0, 2).reshape(P, 8 * E)
    )
    bo1 = np.asarray(bo).astype(bf16).reshape(1, E)
    ones128 = np.ones((P, 1), dtype=bf16)
    sel = np.zeros((P, 8 * P), dtype=np.float32)
    for j in range(8):
        for m in range(P):
            sel[16 * j + 8 * (m // D), j * P + m] = 1.0
    sel = sel.astype(bf16)
    return [
        {
            "x": xs_pre[j],
            "Wv": wv_pre,
            "Wo": wo_pre,
            "bo1": bo1,
            "ones128": ones128,
            "sel": sel,
        }
        for j in range(NCORES)
    ]


def kernel(x, Wq=None, Wk=None, Wv=None, Wo=None, bo=None, **_unused):
    from concourse.bass_utils import run_bass_kernel_spmd

    global _NC_CACHE
    if _NC_CACHE is None:
        _NC_CACHE = build_nc()
    nc = _NC_CACHE

    in_maps = make_in_maps(x, Wv, Wo, bo)
    res = run_bass_kernel_spmd(nc, in_maps, core_ids=list(range(NCORES))).results
    return np.stack(
        [np.asarray(res[j]["out"]).astype(np.float32) for j in range(NCORES)], axis=0
    )


# revision 32
# speedup vs baseline: 1.1599x; 1.1458x over previous
"""Trainium2 Bass kernel for nn_MultiHeadAttention_79130477461654.

The reference einsum "nhqk,nhvd->nhqd" contracts k and v independently, so
out = (sum_k softmax(energy))*(sum_s v) = broadcast(sum_s v) since softmax
rows sum to 1.  With v = split_heads(x @ Wv) and the reference's direct
(n,h,q,d)->(n,s,e) reshape, the full output reduces to

    xs[n]    = sum_s x[n,s,:]                       (1024,)
    Sfull[n] = xs[n] @ Wv                           (1024,)
    T[n,h,:] = tile16(Sfull[n][64h:64h+64]) @ Wo + bo   (16, 1024)
    out[n, 64h+r, :] = T[n,h,:]   for r in 0..63

Sharding: data parallel over batch N=8, one batch per core; Wv/Wo
replicated.  All tensors ship in bf16 and are pre-shuffled on the host
into the exact SBUF tile layouts (pure layout transform: every DMA line
is >=4KB contiguous, ~420GB/s).  All arithmetic runs on-device with f32
PSUM accumulation; end-to-end rel err ~5e-3 vs the 2e-2 gate.  Per-core
HBM traffic: x 2MB + Wv 2MB + Wo 2MB in, out 2MB.

Pipeline: x streams first on both HWDGE queues and reduces on DVE
chasing the DMAs while dummy matmuls warm the PE HAM clock gate; S
accumulates 1024-wide matmuls in wv-arrival order; dbl128[p, 8h+j] =
Sfull[64h + p%64] makes T128[8h+j, :] = T[h, :] land on all 128
partitions; the output expands to full 1024-row form with one-hot sel
matmuls (PSUM alternating between the psO pool and the retired psT
bank) and PSUM->SBUF copies alternating DVE/ACT, then streams out as
four 512KB DMAs.
"""

import numpy as np

N, S, E, H, D = 8, 1024, 1024, 16, 64
NCORES = 8
P = 128  # partitions
REP = P // H  # 8 copies of each head row


def build_nc():
    import concourse.bacc as bacc
    import concourse.mybir as mybir
    from concourse.tile import TileContext

    F32 = mybir.dt.float32
    BF16 = mybir.dt.bfloat16
    nc = bacc.Bacc("TRN2", target_bir_lowering=False, debug=False)

    xd = nc.declare_dram_parameter("x", [P, 8 * E], BF16, isOutput=False)
    wvd = nc.declare_dram_parameter("Wv", [P, 8 * E], BF16, isOutput=False)
    wod = nc.declare_dram_parameter("Wo", [P, 8 * E], BF16, isOutput=False)
    bod = nc.declare_dram_parameter("bo1", [1, E], BF16, isOutput=False)
    ones128d = nc.declare_dram_parameter("ones128", [P, 1], BF16, isOutput=False)
    seld = nc.declare_dram_parameter("sel", [P, 8 * P], BF16, isOutput=False)
    outd = nc.declare_dram_parameter("out", [S, E], BF16, isOutput=True)

    # two HWDGE queues: SP (sync) and ACT (scalar)
    dmae = [nc.sync, nc.scalar]
    # wv/wo quarter-transfer completion order given the queue layout below:
    # q0 gets chunk-pairs (0,1) then (2,3); q1 gets (4,5) then (6,7).
    KORDER = [0, 1, 4, 5, 2, 3, 6, 7]

    with TileContext(nc) as tc:
        with (
            tc.tile_pool(name="xin", bufs=4) as xp,
            tc.tile_pool(name="wv", bufs=4) as wvp,
            tc.tile_pool(name="wo", bufs=4) as wop,
            tc.tile_pool(name="small", bufs=1) as sp,
            tc.tile_pool(name="psA", bufs=1, space="PSUM") as psA,
            tc.tile_pool(name="psS", bufs=1, space="PSUM") as psS,
            tc.tile_pool(name="psT", bufs=1, space="PSUM") as psT,
            tc.tile_pool(name="psO", bufs=1, space="PSUM") as psO,
        ):
            with tc.high_priority():
                ones_sb = sp.tile([P, 1], BF16)
                dmae[0].dma_start(out=ones_sb[:], in_=ones128d[:])
                bo_sb = sp.tile([1, E], BF16)
                dmae[1].dma_start(out=bo_sb[:], in_=bod[:])

                # ---- x: 4 x 512KB transfers, heads of both queues (pinned
                #      first via priority).  Transfer i holds rows
                #      [256i, 256i+256): line p = rows 256i+2p(+1).
                xt = []
                for i in range(4):
                    t = xp.tile([P, 2 * E], BF16)
                    dmae[i % 2].dma_start(
                        out=t[:], in_=xd[:, i * 2 * E : (i + 1) * 2 * E]
                    )
                    xt.append(t)
            # Wv/Wo: 4 x 512KB each, chunk-pairs (0,1)/(2,3) on q0 and
            # (4,5)/(6,7) on q1, so halves of both land early; consumers
            # run in KORDER.  wv tile q holds K-chunks 2q, 2q+1 of Wv;
            # wo tile q likewise (full-width rows).
            wvt, wot = [], []
            for q in range(4):
                pair = [0, 2, 1, 3][q]  # issue order: (0,1) q0, (4,5) q1, ...
                t = wvp.tile([P, 2 * E], BF16, tag=f"wv{pair}")
                dmae[q % 2].dma_start(
                    out=t[:], in_=wvd[:, pair * 2 * E : (pair + 1) * 2 * E]
                )
                wvt.append((pair, t))
            for q in range(4):
                pair = [0, 2, 1, 3][q]
                t = wop.tile([P, 2 * E], BF16, tag=f"wo{pair}")
                dmae[q % 2].dma_start(
                    out=t[:], in_=wod[:, pair * 2 * E : (pair + 1) * 2 * E]
                )
                wot.append((pair, t))
            sel_sb = sp.tile([P, 8 * P], BF16, tag="sel")
            dmae[0].dma_start(out=sel_sb[:], in_=seld[:])
            wvmap = {pair: t for pair, t in wvt}
            womap = {pair: t for pair, t in wot}

            # ---- DVE reduction tree over the 8 row-chunks of x (bf16 2x)
            af = []
            for i in range(4):
                t = sp.tile([P, E], BF16, tag=f"af{i}")
                nc.vector.tensor_add(t[:], xt[i][:, 0:E], xt[i][:, E : 2 * E])
                af.append(t)

            # ---- PE warm-up: dummy 256-wide matmuls keep the HAM clock
            #      gate at 2.4 GHz until S starts; the first dozen feed off
            #      the ones column, the rest off the early af0 tile so the
            #      PE FIFO never stalls on a late x transfer.
            ps_warm = psA.tile([1, 256], F32, tag="psw")

            def warm(rhs):
                nc.tensor.matmul(ps_warm[0:1, :], ones_sb[:], rhs, start=True, stop=True)

            for _ in range(12):
                warm(ones_sb[:, 0:1].to_broadcast((P, 256)))
            for w in range(22):
                warm(af[0][:, (w % 4) * 256 : (w % 4) * 256 + 256])
            ones_r = sp.tile([1, P], BF16, tag="onesr")
            nc.vector.tensor_copy(ones_r[:], ones_sb[0:1, 0:1].to_broadcast((1, P)))

            a01 = sp.tile([P, E], BF16, tag="a01")
            nc.vector.tensor_add(a01[:], af[0][:], af[1][:])
            a23 = sp.tile([P, E], BF16, tag="a23")
            nc.vector.tensor_add(a23[:], af[2][:], af[3][:])

            # ---- xsT[p, c] = xs[128c + p]: PE partition reduction; the
            #      final a01+a23 add is fused into the PSUM accumulation.
            ps_xsT = psA.tile([P, 8], F32, tag="psa")
            for c in range(8):
                nc.tensor.matmul(
                    ps_xsT[:, c : c + 1],
                    a01[:, c * P : (c + 1) * P],
                    ones_sb[:],
                    start=True,
                    stop=False,
                )
                nc.tensor.matmul(
                    ps_xsT[:, c : c + 1],
                    a23[:, c * P : (c + 1) * P],
                    ones_sb[:],
                    start=False,
                    stop=True,
                )
            xsT_b = sp.tile([P, 8], BF16, tag="xsT")
            nc.vector.tensor_copy(xsT_b[:], ps_xsT[:])

            # ---- Sfull row (1, 1024) = xs @ Wv: 1024-wide matmuls in
            #      wv-arrival order.
            ps_S = psS.tile([1, E], F32, tag="pss")
            for idx, k in enumerate(KORDER):
                for half in range(2):
                    sl = slice(half * 512, half * 512 + 512)
                    nc.tensor.matmul(
                        ps_S[0:1, sl],
                        xsT_b[:, k : k + 1],
                        wvmap[k // 2][
                            :, (k % 2) * E + half * 512 : (k % 2) * E + half * 512 + 512
                        ],
                        start=(idx == 0),
                        stop=(idx == 7),
                        skip_group_check=True,
                    )
            srow_b = sp.tile([1, E], BF16, tag="srow")
            nc.vector.tensor_copy(srow_b[0:1, 0:512], ps_S[0:1, 0:512])
            nc.scalar.copy(out=srow_b[0:1, 512:E], in_=ps_S[0:1, 512:E])
            # filler warmups bridge the srow-copy PE gap
            for w in range(4):
                warm(af[0][:, w * 256 : w * 256 + 256])

            # ---- dbl[p, h] = Sfull[64h + p%64] (N=1 matmuls, disjoint
            #      start/stop groups; heads 0-7 chase the DVE srow half),
            #      then replicate each head column 8x.
            ps_dbl = psA.tile([P, H], F32, tag="psa")
            for h in range(H):
                for q in range(2):
                    nc.tensor.matmul(
                        ps_dbl[q * D : (q + 1) * D, h : h + 1],
                        srow_b[0:1, h * D : (h + 1) * D],
                        ones_sb[0:1, 0:1],
                        start=True,
                        stop=True,
                    )
            dbl_b = sp.tile([P, H], BF16, tag="dbl")
            nc.vector.tensor_copy(dbl_b[:], ps_dbl[:])
            dbl128 = sp.tile([P, P], BF16, tag="dbl128")
            nc.vector.tensor_copy(
                dbl128[:].rearrange("p (h j) -> p h j", j=REP),
                dbl_b[:, :, None].to_broadcast((P, H, REP)),
            )

            # ---- T128[8h+j, :] = T[h, :] = dbl128.T @ Wo + bo: 1024-wide
            #      matmuls in wo-arrival order; bias opens the group.
            ps_T = psT.tile([P, E], F32, tag="pst")
            for half in range(2):
                sl = slice(half * 512, half * 512 + 512)
                nc.tensor.matmul(
                    ps_T[:, sl],
                    ones_r[:],
                    bo_sb[0:1, sl],
                    start=True,
                    stop=False,
                    skip_group_check=True,
                )
            for idx, k in enumerate(KORDER):
                for half in range(2):
                    sl = slice(half * 512, half * 512 + 512)
                    nc.tensor.matmul(
                        ps_T[:, sl],
                        dbl128[:],
                        womap[k // 2][
                            :, (k % 2) * E + half * 512 : (k % 2) * E + half * 512 + 512
                        ],
                        start=False,
                        stop=(idx == 7),
                        skip_group_check=True,
                    )
            T_sb = sp.tile([P, E], BF16, tag="tsb")
            nc.vector.tensor_copy(T_sb[:, 0:512], ps_T[:, 0:512])
            nc.scalar.copy(out=T_sb[:, 512:E], in_=ps_T[:, 512:E])

            # ---- expansion: out chunk j rows 128j+p = T[2j + p//64] via
            #      one-hot sel matmuls (PSUM alternates psO pool / retired
            #      psT bank); each chunk's PSUM->SBUF copy is half-split
            #      across DVE+ACT and streams out as its own 256KB DMA.
            ob = sp.tile([P, 8 * E], BF16, tag="ob")
            outr = outd.rearrange("(j p) e -> j p e", p=P)
            for j in range(8):
                if j % 2 == 0:
                    po = psO.tile([P, E], F32, tag="pso")
                else:
                    po = psT.tile([P, E], F32, tag="pst")
                for half in range(2):
                    sl = slice(half * 512, half * 512 + 512)
                    nc.tensor.matmul(
                        po[:, sl],
                        sel_sb[:, j * P : (j + 1) * P],
                        T_sb[:, sl],
                        start=True,
                        stop=True,
                    )
                nc.vector.tensor_copy(ob[:, j * E : j * E + 512], po[:, 0:512])
                nc.scalar.copy(out=ob[:, j * E + 512 : (j + 1) * E], in_=po[:, 512:E])
                dmae[0].dma_start(out=outr[j], in_=ob[:, j * E : (j + 1) * E])

    nc.compile()
    return nc


_NC_CACHE = None


def make_in_maps(x, Wv, Wo, bo):
    import ml_dtypes

    bf16 = ml_dtypes.bfloat16
    x = np.asarray(x).astype(bf16)
    Wv = np.asarray(Wv).astype(bf16)
    Wo = np.asarray(Wo).astype(bf16)
    # pre-shuffle into SBUF tile layouts (pure layout transforms):
    # x_pre[p, i*2048 + r*1024 + e] = x[n, 256i + 2p + r, e]
    xs_pre = [
        np.ascontiguousarray(
            x[j].reshape(4, P, 2, E).transpose(1, 0, 2, 3).reshape(P, 8 * E)
        )
        for j in range(NCORES)
    ]
    # w_pre[p, k*1024 + e] = W[128k + p, e]
    wv_pre = np.ascontiguousarray(
        Wv.reshape(8, P, E).transpose(1, 0, 2).reshape(P, 8 * E)
    )
    wo_pre = np.ascontiguousarray(
        Wo.reshape(8, P, E).transpose(1, 0, 2).reshape(P, 8 * E)
    )
    bo1 = np.asarray(bo).astype(bf16).reshape(1, E)
    ones128 = np.ones((P, 1), dtype=bf16)
    sel = np.zeros((P, 8 * P), dtype=np.float32)
    for j in range(8):
        for m in range(P):
            sel[16 * j + 8 * (m // D), j * P + m] = 1.0
    sel = sel.astype(bf16)
    return [
        {
            "x": xs_pre[j],
            "Wv": wv_pre,
            "Wo": wo_pre,
            "bo1": bo1,
            "ones128": ones128,
            "sel": sel,
        }
        for j in range(NCORES)
    ]


def kernel(x, Wq=None, Wk=None, Wv=None, Wo=None, bo=None, **_unused):
    from concourse.bass_utils import run_bass_kernel_spmd

    global _NC_CACHE
    if _NC_CACHE is None:
        _NC_CACHE = build_nc()
    nc = _NC_CACHE

    in_maps = make_in_maps(x, Wv, Wo, bo)
    res = run_bass_kernel_spmd(nc, in_maps, core_ids=list(range(NCORES))).results
    return np.stack(
        [np.asarray(res[j]["out"]).astype(np.float32) for j in range(NCORES)], axis=0
    )
